# revision 2
# baseline (speedup 1.0000x reference)
"""Trainium2 Bass kernel for nn_LocalGreedyLayer (LIF spiking layer).

Computes, for x_seq [T=16, B=512, IN=3072], fc_w [2048,3072], fc_b [2048],
aux_w [10,2048], aux_b [10]:
    cur  = x_seq @ fc_w.T + fc_b            # [T,B,OUT]
    LIF scan (tau=2, v_th=1, hard reset to 0) -> spk_seq [T,B,OUT]
    count = spk_seq.sum(0)                  # [B,OUT]
    logits = count @ aux_w.T + aux_b        # [B,10]

Sharding: data-parallel over batch B across 8 NeuronCores (64 batch rows
per core); weights replicated. Per core the matmul is computed in a
transposed layout curT [OUT, T*64] so OUT lives on SBUF partitions, the
time scan runs on [128, 16*64] tiles (partition = out%128, free =
(out//128, b)), and spikes/counts are emitted transposed; the host
reassembles the full outputs.
"""

import sys

sys.path.insert(0, "/opt/trn_rl_repo")

from contextlib import ExitStack

import numpy as np

import concourse.bass as bass  # noqa: F401  (bass types used via bacc/tile)
import concourse.tile as tile
from concourse import bacc, mybir
from concourse.bass_utils import run_bass_kernel_spmd

T, B, IN, OUT, NCLS = 16, 512, 3072, 2048, 10
NCORES = 8
BL = B // NCORES          # 64 batch rows per core
TB = T * BL               # 1024 columns of curT per core
NK = IN // 128            # 24 contraction chunks
NOC = OUT // 128          # 16 output chunks
NB = 2                    # N-blocks of 512 columns (8 timesteps each)
NBW = TB // NB            # 512
TPB = T // NB             # 8 timesteps per N-block

F32 = mybir.dt.float32
BF16 = mybir.dt.bfloat16

# "fp32": native fp32 matmul (4 cyc/row, exact). "fp32r": reduced-precision
# fast path (1 cyc/row).
MM_MODE = "fp32"

TRACE = False
LAST_RESULTS = None


def _build_nc():
    nc = bacc.Bacc("TRN2", target_bir_lowering=False, debug=False,
                   enable_asserts=False, num_devices=NCORES)

    xT = nc.dram_tensor("xT", [IN, TB], F32, kind="ExternalInput").ap()
    wT = nc.dram_tensor("wT", [IN, OUT], F32, kind="ExternalInput").ap()
    bias = nc.dram_tensor("bias", [128, NOC], F32, kind="ExternalInput").ap()
    auxwT = nc.dram_tensor("auxwT", [OUT, NCLS], F32, kind="ExternalInput").ap()
    auxb = nc.dram_tensor("auxb", [BL, NCLS], F32, kind="ExternalInput").ap()

    spk = nc.dram_tensor("spk", [T, NOC, 128, BL], BF16, kind="ExternalOutput").ap()
    cnt = nc.dram_tensor("cnt", [NOC, 128, BL], F32, kind="ExternalOutput").ap()
    logits = nc.dram_tensor("logits", [BL, NCLS], F32, kind="ExternalOutput").ap()

    mm_dt = F32 if MM_MODE == "fp32" else mybir.dt.float32r

    with tile.TileContext(nc) as tc, ExitStack() as ctx:
        xres = ctx.enter_context(tc.tile_pool(name="xres", bufs=NK))
        wpool = ctx.enter_context(tc.tile_pool(name="w", bufs=NK))
        curpool = ctx.enter_context(tc.tile_pool(name="cur", bufs=NB))
        vpool = ctx.enter_context(tc.tile_pool(name="v", bufs=2))
        spool = ctx.enter_context(tc.tile_pool(name="s", bufs=3))
        cpool = ctx.enter_context(tc.tile_pool(name="cnt", bufs=2))
        mpool = ctx.enter_context(tc.tile_pool(name="misc", bufs=1))
        ppool = ctx.enter_context(tc.tile_pool(name="psum", bufs=4, space="PSUM"))

        # ---- resident loads ----
        xT_t = xT.rearrange("(k p) n -> k p n", p=128)
        x_tiles = []
        for k in range(NK):
            xt = xres.tile([128, TB], mm_dt, tag="x")
            if mm_dt is F32:
                nc.sync.dma_start(xt[:], xT_t[k])
            else:
                nc.gpsimd.dma_start(xt[:], xT_t[k])
            x_tiles.append(xt)

        bias_t = mpool.tile([128, NOC], F32, tag="bias")
        nc.sync.dma_start(bias_t[:], bias[:, :])
        auxw_t = mpool.tile([128, NOC * NCLS], F32, tag="auxw")
        # auxwT [(j p), n] -> tile [p, (j n)]
        nc.sync.dma_start(
            auxw_t[:].rearrange("p (j n) -> p j n", n=NCLS),
            auxwT.rearrange("(j p) n -> p j n", p=128),
        )
        auxb_t = mpool.tile([BL, NCLS], F32, tag="auxb")
        nc.sync.dma_start(auxb_t[:], auxb[:, :])

        # ---- matmul: curT[o, (t,b)] = wT.T @ xT  (pre-halved w/bias) ----
        wT_r = wT.rearrange("(k p) (o m) -> o k p m", p=128, m=128)
        cur_tiles = []
        for nb in range(NB):
            cur = curpool.tile([128, TPB * NOC * BL], F32, tag="cur")
            cur_tiles.append(cur)
            for oc in range(NOC):
                psum = ppool.tile([128, NBW], F32, tag="mm")
                for k in range(NK):
                    wt = wpool.tile([128, 128], mm_dt, tag="w")
                    if mm_dt is F32:
                        nc.sync.dma_start(wt[:], wT_r[oc, k])
                    else:
                        nc.gpsimd.dma_start(wt[:], wT_r[oc, k])
                    nc.tensor.matmul(
                        psum[:], wt[:], x_tiles[k][:, nb * NBW:(nb + 1) * NBW],
                        start=(k == 0), stop=(k == NK - 1),
                    )
                # psum [p, (t_local, b)] -> cur [p, t_local*NOC*BL + oc*BL + b]
                out_ap = cur[:].rearrange(
                    "p (t o b) -> p t o b", t=TPB, o=NOC, b=BL)[:, :, oc, :]
                nc.scalar.activation(
                    out_ap, psum[:].rearrange("p (t b) -> p t b", b=BL),
                    mybir.ActivationFunctionType.Identity,
                    bias=bias_t[:, oc:oc + 1], scale=1.0,
                )

        # ---- LIF scan over t; v layout [p, (oc, b)] ----
        v = vpool.tile([128, NOC * BL], F32, tag="v")
        nc.vector.memset(v[:], 0.0)
        c_acc = cpool.tile([128, NOC * BL], F32, tag="cnt")
        nc.vector.memset(c_acc[:], 0.0)
        for t in range(T):
            nb, tl = divmod(t, TPB)
            c_slice = cur_tiles[nb][:, tl * NOC * BL:(tl + 1) * NOC * BL]
            v2 = vpool.tile([128, NOC * BL], F32, tag="v")
            nc.vector.scalar_tensor_tensor(
                v2[:], v[:], 0.5, c_slice,
                op0=mybir.AluOpType.mult, op1=mybir.AluOpType.add,
            )
            s = spool.tile([128, NOC * BL], BF16, tag="s")
            nc.vector.tensor_scalar(
                s[:], v2[:], 1.0, None, op0=mybir.AluOpType.is_ge,
            )
            c2 = cpool.tile([128, NOC * BL], F32, tag="cnt")
            nc.vector.tensor_tensor(
                c2[:], s[:], c_acc[:], op=mybir.AluOpType.add,
            )
            c_acc = c2
            v3 = vpool.tile([128, NOC * BL], F32, tag="v")
            nc.vector.scalar_tensor_tensor(
                v3[:], v2[:], 1.0, v2[:],
                op0=mybir.AluOpType.is_lt, op1=mybir.AluOpType.mult,
            )
            v = v3
            nc.sync.dma_start(
                spk[t].rearrange("o p b -> p o b"),
                s[:].rearrange("p (o b) -> p o b", b=BL),
            )

        nc.sync.dma_start(
            cnt.rearrange("o p b -> p o b"),
            c_acc[:].rearrange("p (o b) -> p o b", b=BL),
        )

        # ---- logits = count @ aux_w.T + aux_b  ([BL, 10]) ----
        lpsum = ppool.tile([BL, NCLS], F32, tag="lg")
        for j in range(NOC):
            nc.tensor.matmul(
                lpsum[:], c_acc[:, j * BL:(j + 1) * BL],
                auxw_t[:, j * NCLS:(j + 1) * NCLS],
                start=(j == 0), stop=(j == NOC - 1),
            )
        lsb = mpool.tile([BL, NCLS], F32, tag="lgs")
        nc.vector.tensor_tensor(lsb[:], lpsum[:], auxb_t[:], op=mybir.AluOpType.add)
        nc.sync.dma_start(logits, lsb[:])

    nc.compile()
    return nc


_NC = None


def kernel(x_seq, fc_w, fc_b, aux_w, aux_b):
    global _NC, LAST_RESULTS
    if _NC is None:
        _NC = _build_nc()

    wT = np.ascontiguousarray(fc_w.astype(np.float32).T * np.float32(0.5))
    bias = np.ascontiguousarray(
        (fc_b.astype(np.float32) * np.float32(0.5)).reshape(NOC, 128).T)
    auxwT = np.ascontiguousarray(aux_w.astype(np.float32).T)
    auxb = np.ascontiguousarray(
        np.broadcast_to(aux_b.astype(np.float32), (BL, NCLS)))

    in_maps = []
    for c in range(NCORES):
        xs = x_seq[:, c * BL:(c + 1) * BL, :].astype(np.float32)
        xT = np.ascontiguousarray(xs.transpose(2, 0, 1).reshape(IN, TB))
        in_maps.append(
            {"xT": xT, "wT": wT, "bias": bias, "auxwT": auxwT, "auxb": auxb})

    res = run_bass_kernel_spmd(
        _NC, in_maps, core_ids=list(range(NCORES)), trace=TRACE)
    LAST_RESULTS = res

    spk_parts, cnt_parts, log_parts = [], [], []
    for c in range(NCORES):
        r = res.results[c]
        spk_c = np.asarray(r["spk"]).astype(np.float32)
        spk_parts.append(spk_c.transpose(0, 3, 1, 2).reshape(T, BL, OUT))
        cnt_parts.append(
            np.asarray(r["cnt"]).transpose(2, 0, 1).reshape(BL, OUT))
        log_parts.append(np.asarray(r["logits"]))
    spk_seq = np.concatenate(spk_parts, axis=1)
    count = np.concatenate(cnt_parts, axis=0)
    logits = np.concatenate(log_parts, axis=0)
    return spk_seq, count, logits


# revision 6
# speedup vs baseline: 1.8264x; 1.8264x over previous
"""Trainium2 Bass kernel for nn_LocalGreedyLayer (LIF spiking layer).

Computes, for x_seq [T=16, B=512, IN=3072], fc_w [2048,3072], fc_b [2048],
aux_w [10,2048], aux_b [10]:
    cur  = x_seq @ fc_w.T + fc_b            # [T,B,OUT]
    LIF scan (tau=2, v_th=1, hard reset to 0) -> spk_seq [T,B,OUT]
    count = spk_seq.sum(0)                  # [B,OUT]
    logits = count @ aux_w.T + aux_b        # [B,10]

Sharding: data-parallel over batch B across 8 NeuronCores (64 batch rows
per core); weights replicated. Per core the matmul is computed in a
transposed layout curT [OUT, T*64] so OUT lives on SBUF partitions, the
time scan runs on [128, 16*64] tiles (partition = out%128, free =
(out//128, b)), and spikes/counts are emitted transposed; the host
reassembles the full outputs.
"""

import sys

sys.path.insert(0, "/opt/trn_rl_repo")

from contextlib import ExitStack

import numpy as np

import concourse.bass as bass  # noqa: F401  (bass types used via bacc/tile)
import concourse.tile as tile
from concourse import bacc, mybir
from concourse.bass_utils import run_bass_kernel_spmd

T, B, IN, OUT, NCLS = 16, 512, 3072, 2048, 10
NCORES = 8
BL = B // NCORES          # 64 batch rows per core
TB = T * BL               # 1024 columns of curT per core
NK = IN // 128            # 24 contraction chunks
NOC = OUT // 128          # 16 output chunks
NB = 2                    # N-blocks of 512 columns (8 timesteps each)
NBW = TB // NB            # 512
TPB = T // NB             # 8 timesteps per N-block

F32 = mybir.dt.float32
BF16 = mybir.dt.bfloat16

# "fp32": native fp32 matmul (4 cyc/row, exact). "fp32r": reduced-precision
# fast path (1 cyc/row).
MM_MODE = "fp32r"

TRACE = False
LAST_RESULTS = None


def _build_nc():
    nc = bacc.Bacc("TRN2", target_bir_lowering=False, debug=False,
                   enable_asserts=False, num_devices=NCORES)

    xT = nc.dram_tensor("xT", [IN, TB], F32, kind="ExternalInput").ap()
    wT = nc.dram_tensor("wT", [IN, OUT], F32, kind="ExternalInput").ap()
    bias = nc.dram_tensor("bias", [128, NOC], F32, kind="ExternalInput").ap()
    auxwT = nc.dram_tensor("auxwT", [OUT, NCLS], F32, kind="ExternalInput").ap()
    auxb = nc.dram_tensor("auxb", [BL, NCLS], F32, kind="ExternalInput").ap()

    spk = nc.dram_tensor("spk", [T, NOC, 128, BL], BF16, kind="ExternalOutput").ap()
    cnt = nc.dram_tensor("cnt", [NOC, 128, BL], F32, kind="ExternalOutput").ap()
    logits = nc.dram_tensor("logits", [BL, NCLS], F32, kind="ExternalOutput").ap()

    mm_dt = F32 if MM_MODE == "fp32" else mybir.dt.float32r

    with tile.TileContext(nc) as tc, ExitStack() as ctx:
        xres = ctx.enter_context(tc.tile_pool(name="xres", bufs=NK))
        wpool = ctx.enter_context(tc.tile_pool(name="w", bufs=2))
        curpool = ctx.enter_context(tc.tile_pool(name="cur", bufs=NB))
        vpool = ctx.enter_context(tc.tile_pool(name="v", bufs=2))
        spool = ctx.enter_context(tc.tile_pool(name="s", bufs=3))
        cpool = ctx.enter_context(tc.tile_pool(name="cnt", bufs=2))
        mpool = ctx.enter_context(tc.tile_pool(name="misc", bufs=1))
        ppool = ctx.enter_context(tc.tile_pool(name="psum", bufs=4, space="PSUM"))

        # ---- resident loads ----
        xT_t = xT.rearrange("(k p) n -> k p n", p=128)
        x_tiles = []
        for k in range(NK):
            xt = xres.tile([128, TB], mm_dt, tag="x")
            if mm_dt is F32:
                nc.sync.dma_start(xt[:], xT_t[k])
            else:
                nc.gpsimd.dma_start(xt[:], xT_t[k])
            x_tiles.append(xt)

        bias_t = mpool.tile([128, NOC], F32, tag="bias")
        nc.sync.dma_start(bias_t[:], bias[:, :])
        auxw_t = mpool.tile([128, NOC * NCLS], F32, tag="auxw")
        # auxwT [(j p), n] -> tile [p, (j n)]
        nc.sync.dma_start(
            auxw_t[:].rearrange("p (j n) -> p j n", n=NCLS),
            auxwT.rearrange("(j p) n -> p j n", p=128),
        )
        auxb_t = mpool.tile([BL, NCLS], F32, tag="auxb")
        nc.sync.dma_start(auxb_t[:], auxb[:, :])

        # ---- matmul: curT[o, (t,b)] = wT.T @ xT  (pre-halved w/bias) ----
        wT_r = wT.rearrange("(k p) (o m) -> o p k m", p=128, m=128)
        cur_tiles = []
        for nb in range(NB):
            cur = curpool.tile([128, TPB * NOC * BL], F32, tag="cur")
            cur_tiles.append(cur)
            for oc in range(NOC):
                wt = wpool.tile([128, NK * 128], mm_dt, tag="w")
                wt_v = wt[:].rearrange("p (k m) -> p k m", m=128)
                if mm_dt is F32:
                    nc.sync.dma_start(wt_v, wT_r[oc])
                else:
                    nc.gpsimd.dma_start(wt_v, wT_r[oc])
                psum = ppool.tile([128, NBW], F32, tag="mm")
                for k in range(NK):
                    nc.tensor.matmul(
                        psum[:], wt[:, k * 128:(k + 1) * 128],
                        x_tiles[k][:, nb * NBW:(nb + 1) * NBW],
                        start=(k == 0), stop=(k == NK - 1),
                    )
                # psum [p, (t_local, b)] -> cur [p, t_local*NOC*BL + oc*BL + b]
                out_ap = cur[:].rearrange(
                    "p (t o b) -> p t o b", t=TPB, o=NOC, b=BL)[:, :, oc, :]
                nc.scalar.activation(
                    out_ap, psum[:].rearrange("p (t b) -> p t b", b=BL),
                    mybir.ActivationFunctionType.Identity,
                    bias=bias_t[:, oc:oc + 1], scale=1.0,
                )

        # ---- LIF scan over t; v layout [p, (oc, b)] ----
        v = vpool.tile([128, NOC * BL], F32, tag="v")
        nc.vector.memset(v[:], 0.0)
        c_acc = cpool.tile([128, NOC * BL], F32, tag="cnt")
        nc.vector.memset(c_acc[:], 0.0)
        for t in range(T):
            nb, tl = divmod(t, TPB)
            c_slice = cur_tiles[nb][:, tl * NOC * BL:(tl + 1) * NOC * BL]
            v2 = vpool.tile([128, NOC * BL], F32, tag="v")
            nc.vector.scalar_tensor_tensor(
                v2[:], v[:], 0.5, c_slice,
                op0=mybir.AluOpType.mult, op1=mybir.AluOpType.add,
            )
            s = spool.tile([128, NOC * BL], BF16, tag="s")
            nc.vector.tensor_scalar(
                s[:], v2[:], 1.0, None, op0=mybir.AluOpType.is_ge,
            )
            c2 = cpool.tile([128, NOC * BL], F32, tag="cnt")
            nc.vector.tensor_tensor(
                c2[:], s[:], c_acc[:], op=mybir.AluOpType.add,
            )
            c_acc = c2
            v3 = vpool.tile([128, NOC * BL], F32, tag="v")
            nc.vector.scalar_tensor_tensor(
                v3[:], v2[:], 1.0, v2[:],
                op0=mybir.AluOpType.is_lt, op1=mybir.AluOpType.mult,
            )
            v = v3
            nc.sync.dma_start(
                spk[t].rearrange("o p b -> p o b"),
                s[:].rearrange("p (o b) -> p o b", b=BL),
            )

        nc.sync.dma_start(
            cnt.rearrange("o p b -> p o b"),
            c_acc[:].rearrange("p (o b) -> p o b", b=BL),
        )

        # ---- logits = count @ aux_w.T + aux_b  ([BL, 10]) ----
        lpsum = ppool.tile([BL, NCLS], F32, tag="lg")
        for j in range(NOC):
            nc.tensor.matmul(
                lpsum[:], c_acc[:, j * BL:(j + 1) * BL],
                auxw_t[:, j * NCLS:(j + 1) * NCLS],
                start=(j == 0), stop=(j == NOC - 1),
            )
        lsb = mpool.tile([BL, NCLS], F32, tag="lgs")
        nc.vector.tensor_tensor(lsb[:], lpsum[:], auxb_t[:], op=mybir.AluOpType.add)
        nc.sync.dma_start(logits, lsb[:])

    nc.compile()
    return nc


_NC = None


def kernel(x_seq, fc_w, fc_b, aux_w, aux_b):
    global _NC, LAST_RESULTS
    if _NC is None:
        _NC = _build_nc()

    wT = np.ascontiguousarray(fc_w.astype(np.float32).T * np.float32(0.5))
    bias = np.ascontiguousarray(
        (fc_b.astype(np.float32) * np.float32(0.5)).reshape(NOC, 128).T)
    auxwT = np.ascontiguousarray(aux_w.astype(np.float32).T)
    auxb = np.ascontiguousarray(
        np.broadcast_to(aux_b.astype(np.float32), (BL, NCLS)))

    in_maps = []
    for c in range(NCORES):
        xs = x_seq[:, c * BL:(c + 1) * BL, :].astype(np.float32)
        xT = np.ascontiguousarray(xs.transpose(2, 0, 1).reshape(IN, TB))
        in_maps.append(
            {"xT": xT, "wT": wT, "bias": bias, "auxwT": auxwT, "auxb": auxb})

    res = run_bass_kernel_spmd(
        _NC, in_maps, core_ids=list(range(NCORES)), trace=TRACE)
    LAST_RESULTS = res

    spk_parts, cnt_parts, log_parts = [], [], []
    for c in range(NCORES):
        r = res.results[c]
        spk_c = np.asarray(r["spk"]).astype(np.float32)
        spk_parts.append(spk_c.transpose(0, 3, 1, 2).reshape(T, BL, OUT))
        cnt_parts.append(
            np.asarray(r["cnt"]).transpose(2, 0, 1).reshape(BL, OUT))
        log_parts.append(np.asarray(r["logits"]))
    spk_seq = np.concatenate(spk_parts, axis=1)
    count = np.concatenate(cnt_parts, axis=0)
    logits = np.concatenate(log_parts, axis=0)
    return spk_seq, count, logits
